# revision 9
# baseline (speedup 1.0000x reference)
"""Grouped-experts SwiGLU MoE kernel for Trainium2 (8 NeuronCores).

Expert-parallel: core e computes expert e entirely.
  h = silu(x @ gate) * (x @ down); out = h @ up
Per-core Bass/Tile program:
  - x is fed pre-transposed (xT: [D_IN, T]) so the d-contraction matmuls
    read it naturally with d on partitions.
  - Phase 1 produces hT in [j, t] layout (j on partitions), which is exactly
    the stationary layout phase 2 needs -> no on-chip transposes at all.
  - All matmuls run in float32r (full PE rate, fp32-grade accuracy).
  - DMAs are batched via 3D access patterns (rearrange views): big enough to
    amortize the ~625ns serialized HWDGE dispatch, small enough to spread
    HBM traffic evenly and avoid starving the PE around block boundaries.
"""
import sys
if '/opt/trn_rl_repo' not in sys.path:
    sys.path.insert(0, '/opt/trn_rl_repo')
import numpy as np
from concourse import bacc, tile, mybir, bass_utils

E, T, D_IN, D_H = 8, 4096, 2048, 1408
T_B = 1024                 # tokens per block
NK = D_IN // 128           # 16 k-tiles (phase-1 contraction)
NJ = D_H // 128            # 11 j-tiles
NB = T // T_B              # 4 blocks
NC = T_B // 512            # 2 phase-1 moving chunks per block
NTS = T_B // 128           # 8 phase-2 stationary t-subs per block
ND2 = D_IN // 512          # 4 phase-2 output column chunks

_nc_cache = None


def _build():
    f32, f32r = mybir.dt.float32, mybir.dt.float32r
    nc = bacc.Bacc("TRN2", target_bir_lowering=False, debug=False, num_devices=E)
    xT_d = nc.dram_tensor("xT", [D_IN, T], f32r, kind="ExternalInput")
    g_d = nc.dram_tensor("g", [D_IN, D_H], f32r, kind="ExternalInput")
    dn_d = nc.dram_tensor("dn", [D_IN, D_H], f32r, kind="ExternalInput")
    up_d = nc.dram_tensor("up", [D_H, D_IN], f32r, kind="ExternalInput")
    o_d = nc.dram_tensor("o", [T, D_IN], f32, kind="ExternalOutput")
    Silu = mybir.ActivationFunctionType.Silu

    # strided views for batched DMA
    xT_v = xT_d.ap().rearrange("(k p) t -> p k t", p=128)      # [128, NK, T]
    g_v = g_d.ap().rearrange("(k p) j -> p k j", p=128)        # [128, NK, D_H]
    dn_v = dn_d.ap().rearrange("(k p) j -> p k j", p=128)
    up_v = up_d.ap().rearrange("(j p) d -> p j d", p=128)      # [128, NJ, D_IN]

    with tile.TileContext(nc) as tc:
        with tc.tile_pool(name="sb", bufs=1) as pool, \
             tc.tile_pool(name="ws", bufs=2) as wpool, \
             tc.tile_pool(name="ps", bufs=2, space="PSUM") as psum:
            for b in range(NB):
                xt = pool.tile([128, NK, T_B], f32r, tag="x")
                gdt0 = None
                if b == 0:
                    # startup: j=0 weights first, then x in k-quads, so the
                    # first matmuls begin after ~2MB of DMA instead of ~10MB
                    gt0 = wpool.tile([128, NK, 128], f32r, tag="g")
                    dt0 = wpool.tile([128, NK, 128], f32r, tag="d")
                    gdt0 = (gt0, dt0)
                    nc.sync.dma_start(gt0[:], g_v[:, :, 0:128])
                    nc.sync.dma_start(dt0[:], dn_v[:, :, 0:128])
                    for q in range(4):
                        nc.sync.dma_start(
                            xt[:, q*4:(q+1)*4, 0:512],
                            xT_v[:, q*4:(q+1)*4, 0:512])
                    nc.sync.dma_start(
                        xt[:, :, 512:T_B], xT_v[:, :, 512:T_B])
                else:
                    for c in range(NC):
                        t0 = b*T_B + c*512
                        nc.sync.dma_start(
                            xt[:, :, c*512:(c+1)*512], xT_v[:, :, t0:t0+512])
                hts = []
                for j in range(NJ):
                    if j == 0 and gdt0 is not None:
                        gt, dt = gdt0
                    else:
                        gt = wpool.tile([128, NK, 128], f32r, tag="g")
                        dt = wpool.tile([128, NK, 128], f32r, tag="d")
                        nc.sync.dma_start(gt[:], g_v[:, :, j*128:(j+1)*128])
                        nc.sync.dma_start(dt[:], dn_v[:, :, j*128:(j+1)*128])
                    ht = pool.tile([128, T_B], f32r, tag=f"h{j}")
                    for c in range(NC):
                        pg = psum.tile([128, 512], f32, tag="pg")
                        pd = psum.tile([128, 512], f32, tag="pd")
                        xs = [xt[:, k, c*512:(c+1)*512] for k in range(NK)]
                        for k in range(NK):
                            nc.tensor.matmul(pg[:], gt[:, k, :], xs[k],
                                             start=(k == 0), stop=(k == NK-1))
                        for k in range(NK):
                            nc.tensor.matmul(pd[:], dt[:, k, :], xs[k],
                                             start=(k == 0), stop=(k == NK-1))
                        tmp = wpool.tile([128, 512], f32, tag="silu")
                        nc.scalar.activation(tmp[:], pg[:], Silu)
                        nc.vector.tensor_mul(
                            ht[:, c*512:(c+1)*512], tmp[:], pd[:])
                    hts.append(ht)
                for dc in range(ND2):
                    uts = []
                    for j in range(NJ):
                        ut = wpool.tile([128, 512], f32r, tag=f"u{j}")
                        nc.sync.dma_start(
                            ut[:], up_v[:, j, dc*512:(dc+1)*512])
                        uts.append(ut)
                    for ts in range(NTS):
                        po = psum.tile([128, 512], f32, tag="po")
                        for j in range(NJ):
                            nc.tensor.matmul(
                                po[:], hts[j][:, ts*128:(ts+1)*128], uts[j][:],
                                start=(j == 0), stop=(j == NJ-1))
                        ot = wpool.tile([128, 512], f32, tag="ot")
                        nc.any.tensor_copy(ot[:], po[:])
                        r0 = b*T_B + ts*128
                        nc.sync.dma_start(
                            o_d.ap()[r0:r0+128, dc*512:(dc+1)*512], ot[:])
    nc.compile()
    return nc


def _get_nc():
    global _nc_cache
    if _nc_cache is None:
        _nc_cache = _build()
    return _nc_cache


def kernel(x, gate_proj, down_proj, up_proj, tokens_per_expert):
    x = np.asarray(x, dtype=np.float32)
    gate_proj = np.asarray(gate_proj, dtype=np.float32)
    down_proj = np.asarray(down_proj, dtype=np.float32)
    up_proj = np.asarray(up_proj, dtype=np.float32)
    nc = _get_nc()
    in_maps = [{
        "xT": np.ascontiguousarray(x[e].T),
        "g": np.ascontiguousarray(gate_proj[e]),
        "dn": np.ascontiguousarray(down_proj[e]),
        "up": np.ascontiguousarray(up_proj[e]),
    } for e in range(E)]
    res = bass_utils.run_bass_kernel_spmd(nc, in_maps, list(range(E)))
    return np.stack([res.results[e]["o"] for e in range(E)], axis=0)
